# revision 1
# baseline (speedup 1.0000x reference)
"""Trainium2 Bass kernel for nn_ConditionalChannelProjection.

Reference computation (per sample b):
    mod = silu(emb) @ ada_w.T + ada_b          -> shift (C,), scale (C,)
    rms = rsqrt(mean_c(x^2) + eps)             -> per-pixel over channels
    xm  = (x * rms) * (1 + scale) + shift
    y   = selu(conv_w @ xm + conv_b)           (1x1 conv == channel GEMM)

Kernel algebra (per sample), designed to minimize elementwise passes:
    W''[c,o] = lam * (1 + scale[c]) * w[o,c]       (folds modulation scale + selu lam)
    s[o]     = sum_c w[o,c] * shift[c] + conv_b[o]
    g''      = W''^T-gemm(x)                        (PE, unnormalized input)
    t''      = g'' * rms_broadcast                  (DVE tensor_tensor)
    u        = t''/lam + s                          (selu input; t''+lam*s = lam*u)
    rt       = max(t'' + (lam*s - lam*alpha'), -lam*alpha')   = lam*relu(u) - lam*alpha'
    e*       = Exp(t''/lam + (s + ln(lam*alpha')))  = lam*alpha'*e^u    (ACT)
    y        = min(e*, lam*alpha') + rt             (DVE scalar_tensor_tensor)
  where alpha' = selu alpha; exact: y = lam*relu(u) + min(lam*a*e^u, lam*a) - lam*a
    = selu(u).
    rms uses exp(-0.5*ln(m+eps)) so the whole kernel stays within the
    'natural_log_exp_and_others' ACT table set (no Sqrt <-> Exp table thrash).
    Stats sum over channels uses a PE matmul with an all-(1/C) stationary
    matrix, producing mean(x^2) already broadcast across 128 partitions.

Sharding: data-parallel over batch, 4 samples per core, params replicated.
"""

import numpy as np

import concourse.bass as bass
import concourse.bacc as bacc
import concourse.tile as tile
import concourse.mybir as mybir
from concourse import bass_utils
from concourse.masks import make_identity
from concourse.alu_op_type import AluOpType as Op

# The stock act-table insertion pass greedily picks the first table set
# containing each activation function, which alternates exp_and_others /
# natural_log for this kernel's Exp+Ln mix -> 65 table reloads at 1.3us
# each.  Blank every set except natural_log_exp_and_others (preserving
# list positions, which are the act_func_set_ids walrus expects) so both
# functions resolve to the one covering set and a single load is emitted.
import concourse.bacc as _bacc_mod
import concourse.hw_specs as _hw_specs

_ORIG_GET_TABLES = _hw_specs.get_activation_tables
_KEEP_TABLE = "natural_log_exp_and_others"


def _patched_get_tables(arch):
    tables = _ORIG_GET_TABLES(arch)
    return {name: (funcs if name == _KEEP_TABLE else set())
            for name, funcs in tables.items()}


_bacc_mod.get_activation_tables = _patched_get_tables

AF = mybir.ActivationFunctionType
F32 = mybir.dt.float32

B, C, H, W, E = 32, 256, 64, 64, 1024
HW = H * W                    # 4096 pixels per sample
NCORES = 8
BL = B // NCORES              # 4 samples per core
NCHUNK = 8
CH = HW // NCHUNK             # 512 pixels per chunk (= one PSUM bank of f32)
KT = C // 128                 # 2 channel tiles
EPS = 1e-6

LAM = 1.0507009873554804934193349852946
ALPHA = 1.6732632423543772848170429916717
LA = LAM * ALPHA
LN_LA = float(np.log(LA))

# float32r: same fp32 bits, but the PE runs 1 cycle/row (vs 4 for strict
# fp32) at reduced multiply precision.  Used for the main GEMM + x^2 stats.
USE_F32R = True
F32R = mybir.dt.float32r


def _mmcast(ap):
    return ap.bitcast(F32R) if USE_F32R else ap


def _build_program(reps=1):
    nc = bacc.Bacc("TRN2", target_bir_lowering=False, debug=False,
                   num_devices=NCORES)

    x_d = nc.dram_tensor("x", (BL, C, HW), F32, kind="ExternalInput")
    emb_d = nc.dram_tensor("emb", (BL, E), F32, kind="ExternalInput")
    adaw_d = nc.dram_tensor("ada_w", (2 * C, E), F32, kind="ExternalInput")
    adab_d = nc.dram_tensor("ada_b", (2 * C,), F32, kind="ExternalInput")
    convw_d = nc.dram_tensor("conv_w", (C, C), F32, kind="ExternalInput")
    convb_d = nc.dram_tensor("conv_b", (C,), F32, kind="ExternalInput")
    y_d = nc.dram_tensor("y", (BL, C, HW), F32, kind="ExternalOutput")

    with tile.TileContext(nc) as tc:
        for _ in range(reps):
            _kernel(nc, tc, x_d, emb_d, adaw_d, adab_d, convw_d, convb_d, y_d)

    nc.compile()
    return nc


def _kernel(nc, tc, x_d, emb_d, adaw_d, adab_d, convw_d, convb_d, y_d):
    from contextlib import ExitStack
    ctx = ExitStack()
    with ctx:
        consts = ctx.enter_context(tc.tile_pool(name="consts", bufs=1))
        params = ctx.enter_context(tc.tile_pool(name="params", bufs=1))
        spool = ctx.enter_context(tc.tile_pool(name="spool", bufs=2))
        xpool = ctx.enter_context(tc.tile_pool(name="xpool", bufs=2))
        wpool = ctx.enter_context(tc.tile_pool(name="wpool", bufs=2))
        cpool = ctx.enter_context(tc.tile_pool(name="cpool", bufs=3))
        tpool = ctx.enter_context(tc.tile_pool(name="tpool", bufs=3))
        ypool = ctx.enter_context(tc.tile_pool(name="ypool", bufs=3))

        # ---- constants -------------------------------------------------
        ident = consts.tile([128, 128], F32)
        make_identity(nc, ident[:])
        onesC0 = consts.tile([128, 128], F32)
        nc.gpsimd.memset(onesC0[:], 1.0 / C)
        onesC = consts.tile([128, 128], F32)
        nc.vector.tensor_copy(_mmcast(onesC[:]), onesC0[:])
        zero_col = consts.tile([128, 1], F32)
        nc.gpsimd.memset(zero_col[:], 0.0)
        eps_col = consts.tile([128, 1], F32)
        nc.gpsimd.memset(eps_col[:], EPS)

        # ---- parameter prep (runs once, overlapped with first x DMA) ---
        with tc.tile_pool(name="prep", bufs=1) as prep, \
             tc.tile_pool(name="psum_p", bufs=2, space="PSUM") as psum_p:

            # conv_w -> wT[c_part, k_tile, o]  (transposed via PE)
            cw = prep.tile([128, KT, C], F32)
            nc.sync.dma_start(
                cw[:], convw_d.ap().rearrange("(ot op) c -> op ot c", op=128))
            wT = params.tile([128, KT, C], F32)
            for k in range(KT):
                for m in range(KT):
                    pt = psum_p.tile([128, 128], F32, tag="pp")
                    nc.tensor.transpose(
                        pt[:], cw[:, m, k * 128:(k + 1) * 128], ident[:])
                    nc.vector.tensor_copy(wT[:, k, m * 128:(m + 1) * 128],
                                          pt[:])

            # silu(emb) via exp/reciprocal (keeps ACT table = ln/exp set)
            embt = prep.tile([BL, E], F32)
            nc.sync.dma_start(embt[:], emb_d.ap())
            sig = prep.tile([BL, E], F32)
            nc.scalar.activation(sig[:], embt[:], AF.Exp, bias=zero_col[:BL],
                                 scale=-1.0)
            nc.vector.tensor_scalar_add(sig[:], sig[:], 1.0)
            nc.vector.reciprocal(sig[:], sig[:])
            semb = prep.tile([BL, E], F32)
            nc.vector.tensor_mul(semb[:], embt[:], sig[:])

            # silu(emb)^T -> sembT[e_part, e_tile, b]
            sembT = params.tile([128, E // 128, BL], F32)
            for et in range(E // 128):
                pt2 = psum_p.tile([128, BL], F32, tag="pp")
                nc.tensor.transpose(
                    pt2[:], semb[:, et * 128:(et + 1) * 128], ident[:BL, :BL])
                nc.vector.tensor_copy(sembT[:, et, :], pt2[:])

            # bias columns
            adab = params.tile([128, 2 * C // 128], F32)
            nc.sync.dma_start(
                adab[:], adab_d.ap().rearrange("(jt jp) -> jp jt", jp=128))
            convb = params.tile([128, KT], F32)
            nc.sync.dma_start(
                convb[:], convb_d.ap().rearrange("(ot op) -> op ot", op=128))

            # ada_w -> adaT (PE transpose) then
            # modT[jt][j_part, b] = ada_w^T-gemm(silu(emb)) + ada_b.
            # Scale rows (jt 2,3) first: they gate wpp and the first GEMM.
            # Per-jt tiles so each mod matmul waits only on its own
            # transposes; evacuations alternate DVE/ACT (both idle here).
            adaw = prep.tile([128, 2 * C // 128, E], F32)
            nc.sync.dma_start(
                adaw[:], adaw_d.ap().rearrange("(jt jp) e -> jp jt e", jp=128))
            modT = []
            for jt in range(2 * C // 128):
                mt = params.tile([128, BL], F32, tag=f"modT{jt}")
                modT.append(mt)
            for jt in (2, 3, 0, 1):
                at = prep.tile([128, E // 128, 128], F32, tag=f"adaT{jt}")
                for et in range(E // 128):
                    pt = psum_p.tile([128, 128], F32, tag="pp")
                    nc.tensor.transpose(
                        pt[:], adaw[:, jt, et * 128:(et + 1) * 128], ident[:])
                    if et % 2 == 0:
                        nc.vector.tensor_copy(at[:, et, :], pt[:])
                    else:
                        nc.scalar.copy(at[:, et, :], pt[:])
                pmm = psum_p.tile([128, BL], F32, tag="pm")
                for et in range(E // 128):
                    nc.tensor.matmul(
                        pmm[:], at[:, et, :], sembT[:, et, :],
                        start=(et == 0), stop=(et == E // 128 - 1))
                nc.vector.tensor_scalar_add(
                    modT[jt][:], pmm[:], adab[:, jt:jt + 1])

            # per-kernel fused bias constants
            #   cbl[o] = lam*conv_b - lam*alpha   (for rt bias)
            #   cbe[o] = conv_b + ln(lam*alpha)   (for e* bias)
            cbl = params.tile([128, KT], F32)
            nc.vector.tensor_scalar(cbl[:], convb[:], LAM, -LA, Op.mult, Op.add)
            cbe = params.tile([128, KT], F32)
            nc.vector.tensor_scalar_add(cbe[:], convb[:], LN_LA)

        # main-loop PSUM pools created after the prep PSUM pool releases
        # budget (8 banks): pm 1x2 + pg 2x2 + ps 1 = 7
        psum_m = ctx.enter_context(
            tc.tile_pool(name="psum_m", bufs=1, space="PSUM"))
        psum_g = ctx.enter_context(
            tc.tile_pool(name="psum_g", bufs=2, space="PSUM"))
        psum_s = ctx.enter_context(
            tc.tile_pool(name="psum_s", bufs=1, space="PSUM"))

        # ---- main loop -------------------------------------------------
        NPAIR = NCHUNK // 2
        PW = 2 * CH  # 1024 pixels per chunk-pair
        for b in range(BL):
            # load x sample as [c_part, k_tile, pixel], one tile+DMA per
            # chunk-pair so early compute overlaps the rest of the load
            x_b = x_d.ap()[b].rearrange("(kt kp) w -> kp kt w", kp=128)
            xs = []
            for g in range(NPAIR):
                xg = xpool.tile([128, KT, 2, CH], F32, tag=f"xs{g}")
                nc.sync.dma_start(
                    _mmcast(xg[:]),
                    _mmcast(x_b[:, :, bass.ts(g, PW)].rearrange(
                        "kp kt (c w) -> kp kt c w", c=2)))
                xs.append(xg)

            # W''[c_part, k, o] = lam*(1+scale[c]) * wT
            sc = spool.tile([128, KT], F32, tag="sc")
            for k in range(KT):
                nc.vector.tensor_scalar(
                    sc[:, k:k + 1], modT[KT + k][:, b:b + 1],
                    1.0, LAM, Op.add, Op.mult)
            wpp = wpool.tile([128, KT, C], F32)
            for k in range(KT):
                nc.gpsimd.tensor_scalar_mul(
                    _mmcast(wpp[:, k, :]), wT[:, k, :], sc[:, k:k + 1])

            # s[o] matvec + fused bias columns
            br = spool.tile([128, KT], F32, tag="br")
            be = spool.tile([128, KT], F32, tag="be")
            for o in range(KT):
                ps = psum_s.tile([128, 1], F32, tag="ps")
                for k in range(KT):
                    nc.tensor.matmul(
                        ps[:], wT[:, k, o * 128:(o + 1) * 128],
                        modT[k][:, b:b + 1],
                        start=(k == 0), stop=(k == KT - 1))
                nc.vector.tensor_scalar(
                    br[:, o:o + 1], ps[:], LAM, cbl[:, o:o + 1],
                    Op.mult, Op.add)
                nc.vector.tensor_scalar(
                    be[:, o:o + 1], ps[:], cbe[:, o:o + 1], None, Op.add)

            for j in range(NPAIR):
                xj = xs[j]

                # x^2 on GPSIMD (split by chunk-half to cut pipeline latency)
                # mean(x^2) broadcast to 128 partitions (PE); one matmul
                # per (k, chunk-half) since fp32 moving dim caps at 512
                xsq = cpool.tile([128, KT, 2, CH], F32, tag="xsq")
                pm = psum_m.tile([128, 2, CH], F32)
                for c in range(2):
                    nc.gpsimd.tensor_tensor(_mmcast(xsq[:, :, c, :]),
                                            xj[:, :, c, :],
                                            xj[:, :, c, :], Op.mult)
                    for k in range(KT):
                        nc.tensor.matmul(pm[:, c, :], _mmcast(onesC[:]),
                                         _mmcast(xsq[:, k, c, :]),
                                         start=(k == 0), stop=(k == KT - 1))

                # rms = exp(-0.5*ln(m + eps)), ln result reused in place
                rms = cpool.tile([128, PW], F32, tag="rms")
                nc.scalar.activation(rms[:], pm[:].rearrange("p c w -> p (c w)"),
                                     AF.Ln, bias=eps_col[:], scale=1.0)
                nc.scalar.activation(rms[:], rms[:], AF.Exp, bias=zero_col[:],
                                     scale=-0.5)

                for o in range(KT):
                    pg = psum_g.tile([128, 2, CH], F32, tag="pg")
                    for c in range(2):
                        for k in range(KT):
                            nc.tensor.matmul(
                                pg[:, c, :],
                                _mmcast(wpp[:, k, o * 128:(o + 1) * 128]),
                                _mmcast(xj[:, k, c, :]),
                                start=(k == 0), stop=(k == KT - 1))
                    pgf = pg[:].rearrange("p c w -> p (c w)")

                    # t'' = g'' * rms   (DVE)
                    tt = tpool.tile([128, PW], F32, tag="tt")
                    nc.vector.tensor_mul(tt[:], pgf, rms[:])

                    # rt = max(t'' + (lam*s - lam*a), -lam*a)  (DVE)
                    rt = tpool.tile([128, PW], F32, tag="rt")
                    nc.vector.tensor_scalar(
                        rt[:], tt[:], br[:, o:o + 1], -LA, Op.add, Op.max)

                    # e* = Exp(t''/lam + (s + ln(lam*a)))   (ACT)
                    es = tpool.tile([128, PW], F32, tag="es")
                    nc.scalar.activation(es[:], tt[:], AF.Exp,
                                         bias=be[:, o:o + 1], scale=1.0 / LAM)

                    # y = min(e*, lam*a) + rt   (DVE)
                    yo = ypool.tile([128, PW], F32)
                    nc.vector.scalar_tensor_tensor(
                        yo[:], es[:], LA, rt[:], Op.min, Op.add)

                    nc.sync.dma_start(
                        y_d.ap()[b][o * 128:(o + 1) * 128, bass.ts(j, PW)],
                        yo[:])


_program_cache = None


def _get_program():
    global _program_cache
    if _program_cache is None:
        _program_cache = _build_program()
    return _program_cache


def kernel(x, emb, ada_w, ada_b, conv_w, conv_b):
    nc = _get_program()
    x = np.ascontiguousarray(np.asarray(x, dtype=np.float32)).reshape(B, C, HW)
    emb = np.ascontiguousarray(np.asarray(emb, dtype=np.float32))
    ada_w = np.ascontiguousarray(np.asarray(ada_w, dtype=np.float32))
    ada_b = np.ascontiguousarray(np.asarray(ada_b, dtype=np.float32))
    conv_w = np.ascontiguousarray(np.asarray(conv_w, dtype=np.float32))
    conv_b = np.ascontiguousarray(np.asarray(conv_b, dtype=np.float32))

    in_maps = []
    for c in range(NCORES):
        sl = slice(c * BL, (c + 1) * BL)
        in_maps.append({
            "x": x[sl],
            "emb": emb[sl],
            "ada_w": ada_w,
            "ada_b": ada_b,
            "conv_w": conv_w,
            "conv_b": conv_b,
        })

    res = bass_utils.run_bass_kernel_spmd(
        nc, in_maps, core_ids=list(range(NCORES)))
    y = np.concatenate([r["y"].reshape(BL, C, H, W) for r in res.results],
                       axis=0)
    return y



# revision 23
# speedup vs baseline: 1.0831x; 1.0831x over previous
"""Trainium2 Bass kernel for nn_ConditionalChannelProjection.

Reference computation (per sample b):
    mod = silu(emb) @ ada_w.T + ada_b          -> shift (C,), scale (C,)
    rms = rsqrt(mean_c(x^2) + eps)             -> per-pixel over channels
    xm  = (x * rms) * (1 + scale) + shift
    y   = selu(conv_w @ xm + conv_b)           (1x1 conv == channel GEMM)

Kernel algebra (per sample), designed to minimize elementwise passes:
    W''[c,o] = lam * (1 + scale[c]) * w[o,c]       (folds modulation scale + selu lam)
    s[o]     = sum_c w[o,c] * shift[c] + conv_b[o]
    g''      = W''^T-gemm(x)                        (PE, unnormalized input)
    t''      = g'' * rms_broadcast                  (DVE tensor_tensor)
    u        = t''/lam + s                          (selu input; t''+lam*s = lam*u)
    rt       = max(t'' + (lam*s - lam*alpha'), -lam*alpha')   = lam*relu(u) - lam*alpha'
    e*       = Exp(t''/lam + (s + ln(lam*alpha')))  = lam*alpha'*e^u    (ACT)
    y        = min(e*, lam*alpha') + rt             (DVE scalar_tensor_tensor)
  where alpha' = selu alpha; exact: y = lam*relu(u) + min(lam*a*e^u, lam*a) - lam*a
    = selu(u).
    rms uses exp(-0.5*ln(m+eps)) so the whole kernel stays within the
    'natural_log_exp_and_others' ACT table set (no Sqrt <-> Exp table thrash).
    Stats sum over channels uses a PE matmul with an all-(1/C) stationary
    matrix, producing mean(x^2) already broadcast across 128 partitions.

Sharding: data-parallel over batch, 4 samples per core, params replicated.
"""

import numpy as np

import concourse.bass as bass
import concourse.bacc as bacc
import concourse.tile as tile
import concourse.mybir as mybir
from concourse import bass_utils
from concourse.masks import make_identity
from concourse.alu_op_type import AluOpType as Op

# The stock act-table insertion pass greedily picks the first table set
# containing each activation function, which alternates exp_and_others /
# natural_log for this kernel's Exp+Ln mix -> 65 table reloads at 1.3us
# each.  Blank every set except natural_log_exp_and_others (preserving
# list positions, which are the act_func_set_ids walrus expects) so both
# functions resolve to the one covering set and a single load is emitted.
import concourse.bacc as _bacc_mod
import concourse.hw_specs as _hw_specs

_ORIG_GET_TABLES = _hw_specs.get_activation_tables
_KEEP_TABLE = "natural_log_exp_and_others"


def _patched_get_tables(arch):
    tables = _ORIG_GET_TABLES(arch)
    return {name: (funcs if name == _KEEP_TABLE else set())
            for name, funcs in tables.items()}


_bacc_mod.get_activation_tables = _patched_get_tables

AF = mybir.ActivationFunctionType
F32 = mybir.dt.float32

B, C, H, W, E = 32, 256, 64, 64, 1024
HW = H * W                    # 4096 pixels per sample
NCORES = 8
BL = B // NCORES              # 4 samples per core
NCHUNK = 8
CH = HW // NCHUNK             # 512 pixels per chunk (= one PSUM bank of f32)
KT = C // 128                 # 2 channel tiles
EPS = 1e-6

LAM = 1.0507009873554804934193349852946
ALPHA = 1.6732632423543772848170429916717
LA = LAM * ALPHA
LN_LA = float(np.log(LA))

# float32r: same fp32 bits, but the PE runs 1 cycle/row (vs 4 for strict
# fp32) at reduced multiply precision.  Used for the main GEMM + x^2 stats.
USE_F32R = True
F32R = mybir.dt.float32r


def _mmcast(ap):
    return ap.bitcast(F32R) if USE_F32R else ap


def _build_program(reps=1):
    nc = bacc.Bacc("TRN2", target_bir_lowering=False, debug=False,
                   num_devices=NCORES)

    x_d = nc.dram_tensor("x", (BL, C, HW), F32, kind="ExternalInput")
    emb_d = nc.dram_tensor("emb", (BL, E), F32, kind="ExternalInput")
    adaw_d = nc.dram_tensor("ada_w", (2 * C, E), F32, kind="ExternalInput")
    adab_d = nc.dram_tensor("ada_b", (2 * C,), F32, kind="ExternalInput")
    convw_d = nc.dram_tensor("conv_w", (C, C), F32, kind="ExternalInput")
    convb_d = nc.dram_tensor("conv_b", (C,), F32, kind="ExternalInput")
    y_d = nc.dram_tensor("y", (BL, C, HW), F32, kind="ExternalOutput")

    with tile.TileContext(nc) as tc:
        for _ in range(reps):
            _kernel(nc, tc, x_d, emb_d, adaw_d, adab_d, convw_d, convb_d, y_d)

    nc.compile()
    return nc


def _kernel(nc, tc, x_d, emb_d, adaw_d, adab_d, convw_d, convb_d, y_d):
    from contextlib import ExitStack
    ctx = ExitStack()
    with ctx:
        consts = ctx.enter_context(tc.tile_pool(name="consts", bufs=1))
        params = ctx.enter_context(tc.tile_pool(name="params", bufs=1))
        xpool = ctx.enter_context(tc.tile_pool(name="xpool", bufs=2))
        qpool = ctx.enter_context(tc.tile_pool(name="qpool", bufs=2))
        cpool = ctx.enter_context(tc.tile_pool(name="cpool", bufs=3))
        tpool = ctx.enter_context(tc.tile_pool(name="tpool", bufs=3))
        ypool = ctx.enter_context(tc.tile_pool(name="ypool", bufs=3))

        # ---- constants -------------------------------------------------
        ident = consts.tile([128, 128], F32)
        make_identity(nc, ident[:])
        # f32r view of the identity: PE transposes run 1 row/cycle in f32r
        # (vs 4 for strict fp32) and multiply-by-identity is exact.
        identr = consts.tile([128, 128], F32)
        nc.vector.tensor_copy(_mmcast(identr[:]), ident[:])
        onesC0 = consts.tile([128, 128], F32)
        nc.gpsimd.memset(onesC0[:], 1.0 / C)
        onesC = consts.tile([128, 128], F32)
        nc.vector.tensor_copy(_mmcast(onesC[:]), onesC0[:])
        zero_col = consts.tile([128, 1], F32)
        nc.gpsimd.memset(zero_col[:], 0.0)
        eps_col = consts.tile([128, 1], F32)
        nc.gpsimd.memset(eps_col[:], EPS)
        la_col = consts.tile([128, 1], F32)
        nc.gpsimd.memset(la_col[:], LA)

        # ---- parameter prep (runs once, overlapped with first x DMA) ---
        with tc.tile_pool(name="prep", bufs=1) as prep, \
             tc.tile_pool(name="psum_p", bufs=2, space="PSUM") as psum_p:

            # conv_w -> wT[c_part, k_tile, o]  (transposed via PE)
            cw = prep.tile([128, KT, C], F32)
            nc.sync.dma_start(
                _mmcast(cw[:]),
                _mmcast(convw_d.ap().rearrange("(ot op) c -> op ot c", op=128)))
            wT = params.tile([128, KT, C], F32)
            for k in range(KT):
                for m in range(KT):
                    pt = psum_p.tile([128, 128], F32, tag="pp")
                    nc.tensor.transpose(
                        pt[:].bitcast(F32R),
                        _mmcast(cw[:, m, k * 128:(k + 1) * 128]),
                        identr[:].bitcast(F32R))
                    nc.vector.tensor_copy(
                        _mmcast(wT[:, k, m * 128:(m + 1) * 128]), pt[:])

            # silu(emb) via exp/reciprocal (keeps ACT table = ln/exp set)
            embt = prep.tile([BL, E], F32)
            nc.sync.dma_start(embt[:], emb_d.ap())
            sig = prep.tile([BL, E], F32)
            nc.scalar.activation(sig[:], embt[:], AF.Exp, bias=zero_col[:BL],
                                 scale=-1.0)
            nc.vector.tensor_scalar_add(sig[:], sig[:], 1.0)
            nc.vector.reciprocal(sig[:], sig[:])
            semb = prep.tile([BL, E], F32)
            nc.vector.tensor_mul(_mmcast(semb[:]), embt[:], sig[:])

            # silu(emb)^T -> sembT[e_part, e_tile, b]
            sembT = params.tile([128, E // 128, BL], F32)
            for et in range(E // 128):
                pt2 = psum_p.tile([128, BL], F32, tag="pp")
                nc.tensor.transpose(
                    pt2[:].bitcast(F32R),
                    _mmcast(semb[:, et * 128:(et + 1) * 128]),
                    identr[:BL, :BL].bitcast(F32R))
                nc.vector.tensor_copy(_mmcast(sembT[:, et, :]), pt2[:])

            # bias columns
            adab = params.tile([128, 2 * C // 128], F32)
            nc.sync.dma_start(
                adab[:], adab_d.ap().rearrange("(jt jp) -> jp jt", jp=128))
            convb = params.tile([128, KT], F32)
            nc.sync.dma_start(
                convb[:], convb_d.ap().rearrange("(ot op) -> op ot", op=128))

            # ada_w -> adaT (PE transpose) then
            # modT[jt][j_part, b] = ada_w^T-gemm(silu(emb)) + ada_b.
            # Scale rows (jt 2,3) first: they gate wpp and the first GEMM.
            # Per-jt tiles so each mod matmul waits only on its own
            # transposes; evacuations alternate DVE/ACT (both idle here).
            adaw = prep.tile([128, 2 * C // 128, E], F32)
            nc.sync.dma_start(
                _mmcast(adaw[:]),
                _mmcast(adaw_d.ap().rearrange("(jt jp) e -> jp jt e", jp=128)))
            modT = []
            for jt in range(2 * C // 128):
                mt = params.tile([128, BL], F32, tag=f"modT{jt}")
                modT.append(mt)
            for jt in (2, 3, 0, 1):
                at = prep.tile([128, E // 128, 128], F32, tag="adaT")
                for et in range(E // 128):
                    pt = psum_p.tile([128, 128], F32, tag="pp")
                    nc.tensor.transpose(
                        pt[:].bitcast(F32R),
                        _mmcast(adaw[:, jt, et * 128:(et + 1) * 128]),
                        identr[:].bitcast(F32R))
                    if et % 2 == 0:
                        nc.vector.tensor_copy(_mmcast(at[:, et, :]), pt[:])
                    else:
                        nc.scalar.copy(_mmcast(at[:, et, :]), pt[:])
                pmm = psum_p.tile([128, BL], F32, tag="pm")
                for et in range(E // 128):
                    nc.tensor.matmul(
                        pmm[:], _mmcast(at[:, et, :]),
                        _mmcast(sembT[:, et, :]),
                        start=(et == 0), stop=(et == E // 128 - 1))
                nc.vector.tensor_scalar_add(
                    _mmcast(modT[jt][:]), pmm[:], adab[:, jt:jt + 1])

            # per-kernel fused bias constants
            #   cbl[o] = lam*conv_b - lam*alpha   (for rt bias)
            #   cbe[o] = conv_b + ln(lam*alpha)   (for e* bias)
            cbl = params.tile([128, KT], F32)
            nc.vector.tensor_scalar(cbl[:], convb[:], LAM, -LA, Op.mult, Op.add)
            cbe = params.tile([128, KT], F32)
            nc.vector.tensor_scalar_add(cbe[:], convb[:], LN_LA)

            # s[o] matvec for ALL samples at once (stationary wT, moving
            # modT[k] = shift rows, [128, BL]); fused bias columns.
            # br = lam*s - lam*a (o=0 stt path), brz = lam*s (o=1
            # relu-difference path), be = s + ln(lam*a) (exp bias).
            cbz = params.tile([128, KT], F32)
            nc.vector.tensor_scalar_mul(cbz[:], convb[:], LAM)
            br_all = params.tile([128, KT, BL], F32)
            brz_all = params.tile([128, KT, BL], F32)
            be_all = params.tile([128, KT, BL], F32)
            for o in range(KT):
                pso = psum_p.tile([128, BL], F32, tag="pm")
                for k in range(KT):
                    nc.tensor.matmul(
                        pso[:], _mmcast(wT[:, k, o * 128:(o + 1) * 128]),
                        _mmcast(modT[k][:]),
                        start=(k == 0), stop=(k == KT - 1))
                nc.vector.tensor_scalar(
                    br_all[:, o, :], pso[:], LAM, cbl[:, o:o + 1],
                    Op.mult, Op.add)
                nc.vector.tensor_scalar(
                    brz_all[:, o, :], pso[:], LAM, cbz[:, o:o + 1],
                    Op.mult, Op.add)
                nc.vector.tensor_scalar(
                    be_all[:, o, :], pso[:], cbe[:, o:o + 1], None, Op.add)

            # W''[c_part, b, k, o] = lam*(1+scale[c]) * wT for all samples
            sc_all = params.tile([128, KT, BL], F32)
            for k in range(KT):
                nc.vector.tensor_scalar(
                    sc_all[:, k, :], modT[KT + k][:], 1.0, LAM,
                    Op.add, Op.mult)
            wpp_all = params.tile([128, BL, KT, C], F32)
            for b in range(BL):
                for k in range(KT):
                    nc.vector.tensor_scalar_mul(
                        _mmcast(wpp_all[:, b, k, :]), wT[:, k, :],
                        sc_all[:, k, b:b + 1])

        # main-loop PSUM pools created after the prep PSUM pool releases
        # budget (8 banks): pm 2x2 + pg 2x2 = 8
        psum_m = ctx.enter_context(
            tc.tile_pool(name="psum_m", bufs=2, space="PSUM"))
        psum_g = ctx.enter_context(
            tc.tile_pool(name="psum_g", bufs=2, space="PSUM"))

        # ---- main loop -------------------------------------------------
        NPAIR = NCHUNK // 2
        PW = 2 * CH  # 1024 pixels per chunk-pair
        for b in range(BL):
            # load x sample as [c_part, k_tile, pixel], one tile+DMA per
            # chunk-pair so early compute overlaps the rest of the load
            x_b = x_d.ap()[b].rearrange("(kt kp) w -> kp kt w", kp=128)
            xs = []
            for g in range(NPAIR):
                xg = xpool.tile([128, KT, 2, CH], F32, tag=f"xs{g}")
                nc.sync.dma_start(
                    _mmcast(xg[:]),
                    _mmcast(x_b[:, :, bass.ts(g, PW)].rearrange(
                        "kp kt (c w) -> kp kt c w", c=2)))
                xs.append(xg)

            for j in range(NPAIR):
                xj = xs[j]

                # x^2 on GPSIMD (split by chunk-half to cut pipeline latency)
                # mean(x^2) broadcast to 128 partitions (PE); one matmul
                # per (k, chunk-half) since fp32 moving dim caps at 512
                xsq = qpool.tile([128, KT, 2, CH], F32, tag="xsq")
                pm = psum_m.tile([128, 2, CH], F32)
                for c in range(2):
                    nc.gpsimd.tensor_tensor(_mmcast(xsq[:, :, c, :]),
                                            xj[:, :, c, :],
                                            xj[:, :, c, :], Op.mult)
                    for k in range(KT):
                        nc.tensor.matmul(pm[:, c, :], _mmcast(onesC[:]),
                                         _mmcast(xsq[:, k, c, :]),
                                         start=(k == 0), stop=(k == KT - 1))

                # rms = exp(-0.5*ln(m + eps)), ln result reused in place
                rms = cpool.tile([128, PW], F32, tag="rms")
                nc.scalar.activation(rms[:], pm[:].rearrange("p c w -> p (c w)"),
                                     AF.Ln, bias=eps_col[:], scale=1.0)
                nc.scalar.activation(rms[:], rms[:], AF.Exp, bias=zero_col[:],
                                     scale=-0.5)

                for o in range(KT):
                    pg = psum_g.tile([128, 2, CH], F32, tag="pg")
                    for c in range(2):
                        for k in range(KT):
                            nc.tensor.matmul(
                                pg[:, c, :],
                                _mmcast(wpp_all[:, b, k,
                                                o * 128:(o + 1) * 128]),
                                _mmcast(xj[:, k, c, :]),
                                start=(k == 0), stop=(k == KT - 1))
                    pgf = pg[:].rearrange("p c w -> p (c w)")

                    # t'' = g'' * rms   (DVE)
                    tt = tpool.tile([128, PW], F32, tag="tt")
                    nc.vector.tensor_mul(tt[:], pgf, rms[:])

                    # e* = Exp(t''/lam + (s + ln(lam*a)))   (ACT)
                    es = tpool.tile([128, PW], F32, tag="es")
                    nc.scalar.activation(es[:], tt[:], AF.Exp,
                                         bias=be_all[:, o, b:b + 1],
                                         scale=1.0 / LAM)

                    yo = ypool.tile([128, PW], F32)
                    if o == 0:
                        # rt = max(t'' + (lam*s - lam*a), -lam*a)  (DVE)
                        # y  = min(e*, lam*a) + rt                 (DVE stt)
                        rt = tpool.tile([128, PW], F32, tag="rt")
                        nc.vector.tensor_scalar(
                            rt[:], tt[:], br_all[:, o, b:b + 1], -LA,
                            Op.add, Op.max)
                        nc.vector.scalar_tensor_tensor(
                            yo[:], es[:], LA, rt[:], Op.min, Op.add)
                    else:
                        # Equivalent split that gives GPSIMD the combine:
                        # rt' = max(t'' + lam*s, 0)        (DVE ts)
                        # e2  = relu(lam*a - e*)           (ACT, same table)
                        # y   = rt' - e2                   (GPSIMD subtract)
                        rt = tpool.tile([128, PW], F32, tag="rt")
                        nc.vector.tensor_scalar(
                            rt[:], tt[:], brz_all[:, o, b:b + 1], 0.0,
                            Op.add, Op.max)
                        e2 = tpool.tile([128, PW], F32, tag="e2")
                        nc.scalar.activation(e2[:], es[:], AF.Relu,
                                             bias=la_col[:], scale=-1.0)
                        nc.gpsimd.tensor_tensor(
                            yo[:], rt[:], e2[:], Op.subtract)

                    nc.sync.dma_start(
                        y_d.ap()[b][o * 128:(o + 1) * 128, bass.ts(j, PW)],
                        yo[:])


_program_cache = None


def _get_program():
    global _program_cache
    if _program_cache is None:
        _program_cache = _build_program()
    return _program_cache


def kernel(x, emb, ada_w, ada_b, conv_w, conv_b):
    nc = _get_program()
    x = np.ascontiguousarray(np.asarray(x, dtype=np.float32)).reshape(B, C, HW)
    emb = np.ascontiguousarray(np.asarray(emb, dtype=np.float32))
    ada_w = np.ascontiguousarray(np.asarray(ada_w, dtype=np.float32))
    ada_b = np.ascontiguousarray(np.asarray(ada_b, dtype=np.float32))
    conv_w = np.ascontiguousarray(np.asarray(conv_w, dtype=np.float32))
    conv_b = np.ascontiguousarray(np.asarray(conv_b, dtype=np.float32))

    in_maps = []
    for c in range(NCORES):
        sl = slice(c * BL, (c + 1) * BL)
        in_maps.append({
            "x": x[sl],
            "emb": emb[sl],
            "ada_w": ada_w,
            "ada_b": ada_b,
            "conv_w": conv_w,
            "conv_b": conv_b,
        })

    res = bass_utils.run_bass_kernel_spmd(
        nc, in_maps, core_ids=list(range(NCORES)))
    y = np.concatenate([r["y"].reshape(BL, C, H, W) for r in res.results],
                       axis=0)
    return y

